# revision 1
# baseline (speedup 1.0000x reference)
"""Trainium2 Bass kernel for nn_AttentionHead (sparse attention, 8 cores).

Reference computation (per batch b):
    q = x_q @ wq^T ; k = x_k @ wk^T ; v = x_v @ wv^T          # [S, H]
    s = (q @ k^T) / sqrt(H)                                    # [S, S]
    s = where(mask == 0, 0, s)       # multiplicative 0/1 mask BEFORE softmax
    p = softmax(s, axis=-1)          # masked entries contribute exp(0)=1
    out = p @ v                                                # [S, H]

Sharding: 8 cores; core c -> batch c//2, query rows (c%2)*2048 ... +2048.
Each core computes k/v for its whole batch (duplicated within the pair),
so there are no collectives.

Host-side prep (free w.r.t. HW exec time): x_q/x_k/x_v and the weights are
transposed so the contraction dim (d) lands on SBUF partitions without any
on-chip transposes; the mask is pre-cast to bf16 (0.0/1.0 exact) to halve
its DMA traffic.

On-chip per core:
  phase A: qT[h, sq], kT[h, sk] (f32) and v[sk, h] (bf16) projections (f32r
           matmuls, d-chunked PSUM accumulation).
  phase B: per 128-row query tile: s = qT.T @ kT (f32r) -> PSUM; DVE multiply
           by bf16 mask (PSUM->SBUF); ACT exp(x/16) -> bf16 P with fp32 row-sum
           side output (softmax denominator); PE-transpose P (bf16); P^T @ v
           accumulated over all sk into PSUM; normalize by reciprocal row-sum;
           DMA out.
"""

import numpy as np
import ml_dtypes

import concourse.bass as bass
import concourse.mybir as mybir
import concourse.tile as tile
from concourse import bacc
from concourse import masks
from concourse.bass_utils import run_bass_kernel_spmd

F32 = mybir.dt.float32
F32R = mybir.dt.float32r
BF16 = mybir.dt.bfloat16

# Full-problem constants
B, S, DV, H = 4, 4096, 1024, 256
N_CORES = 8
CORES_PER_BATCH = N_CORES // B
SQL = S // CORES_PER_BATCH  # query rows per core


def build_attention_nc(SQL_, SK_, DV_, H_, scale, num_devices=1):
    """Build the per-core Bass graph. All shape params must be multiples of
    the tile sizes used below (SQL_, SK_ % 512 == 0, DV_ % 128 == 0, H_ == 256).
    """
    P = 128
    SKB = 512                     # sk block width (scores matmul free dim)
    DC = DV_ // P                 # d chunks
    NSKB = SK_ // SKB             # sk blocks
    NKC = SK_ // P                # sk chunks of 128
    NSQT = SQL_ // P              # query tiles
    NSQB = SQL_ // SKB            # query blocks of 512 (for qT projection)
    HC = H_ // P                  # h chunks (lhsT M-dim <= 128)

    nc = bacc.Bacc("TRN2", target_bir_lowering=False, debug=False,
                   num_devices=num_devices)

    x_qT = nc.dram_tensor("x_qT", [DV_, SQL_], F32R, kind="ExternalInput").ap()
    x_kT = nc.dram_tensor("x_kT", [DV_, SK_], F32R, kind="ExternalInput").ap()
    x_vT = nc.dram_tensor("x_vT", [DV_, SK_], F32R, kind="ExternalInput").ap()
    mask = nc.dram_tensor("mask", [SQL_, SK_], BF16, kind="ExternalInput").ap()
    wqT = nc.dram_tensor("wqT", [DV_, H_], F32R, kind="ExternalInput").ap()
    wkT = nc.dram_tensor("wkT", [DV_, H_], F32R, kind="ExternalInput").ap()
    wvT = nc.dram_tensor("wvT", [DV_, H_], F32R, kind="ExternalInput").ap()
    out = nc.dram_tensor("out", [SQL_, H_], F32, kind="ExternalOutput").ap()

    with tile.TileContext(nc) as tc:
        with (
            tc.tile_pool(name="weights", bufs=3) as w_pool,
            tc.tile_pool(name="qT", bufs=HC * NSQB) as qT_pool,
            tc.tile_pool(name="kT", bufs=HC * NSKB) as kT_pool,
            tc.tile_pool(name="vsb", bufs=NKC) as v_pool,
            tc.tile_pool(name="ident", bufs=1) as ident_pool,
        ):
            # ---- constants ----
            identity = ident_pool.tile([P, P], BF16)
            masks.make_identity(nc, identity[:])

            # ---- weights: [DV, H] -> SBUF [128, DC, H] ----
            w_sb = {}
            for name, wT in (("q", wqT), ("k", wkT), ("v", wvT)):
                t = w_pool.tile([P, DC, H_], F32R, tag=f"w_{name}")
                nc.sync.dma_start(
                    out=t[:], in_=wT.rearrange("(dc p) h -> p dc h", p=P))
                w_sb[name] = t

            kT_sb = [[None] * NSKB for _ in range(HC)]
            qT_sb = [[None] * NSQB for _ in range(HC)]
            v_sb = [None] * NKC

            # ---- phase A: projections ----
            with (
                tc.tile_pool(name="xT", bufs=2 * DC) as xT_pool,
                tc.tile_pool(name="projpsum", bufs=2, space="PSUM") as proj_psum,
                tc.tile_pool(name="projpsv", bufs=2, space="PSUM") as proj_psum_v,
            ):
                for skb in range(NSKB):
                    # kT[h, sk] (f32): lhsT = w chunk, rhs = x_kT chunk
                    xk = []
                    for dc in range(DC):
                        t = xT_pool.tile([P, SKB], F32R, tag="xT")
                        nc.sync.dma_start(
                            out=t[:], in_=x_kT[dc * P:(dc + 1) * P,
                                               skb * SKB:(skb + 1) * SKB])
                        xk.append(t)
                    for hc in range(HC):
                        ps = proj_psum.tile([P, SKB], F32, tag="proj_kq")
                        for dc in range(DC):
                            nc.tensor.matmul(
                                ps[:],
                                w_sb["k"][:, dc, hc * P:(hc + 1) * P],
                                xk[dc][:],
                                start=(dc == 0), stop=(dc == DC - 1))
                        t = kT_pool.tile([P, SKB], F32R, tag="kT")
                        nc.vector.tensor_copy(t[:], ps[:])
                        kT_sb[hc][skb] = t

                    # v[sk, h] (bf16): lhsT = x_vT chunk, rhs = w_v chunk
                    xv = []
                    for dc in range(DC):
                        t = xT_pool.tile([P, SKB], F32R, tag="xT")
                        nc.sync.dma_start(
                            out=t[:], in_=x_vT[dc * P:(dc + 1) * P,
                                               skb * SKB:(skb + 1) * SKB])
                        xv.append(t)
                    for j in range(SKB // P):
                        kc = skb * (SKB // P) + j
                        ps = proj_psum_v.tile([P, H_], F32, tag="proj_v")
                        for dc in range(DC):
                            nc.tensor.matmul(
                                ps[:],
                                xv[dc][:, j * P:(j + 1) * P],
                                w_sb["v"][:, dc, :],
                                start=(dc == 0), stop=(dc == DC - 1))
                        t = v_pool.tile([P, H_], BF16, tag="v")
                        nc.vector.tensor_copy(t[:], ps[:])
                        v_sb[kc] = t

                # qT[h, sq] (f32)
                for sqb in range(NSQB):
                    xq = []
                    for dc in range(DC):
                        t = xT_pool.tile([P, SKB], F32R, tag="xT")
                        nc.sync.dma_start(
                            out=t[:], in_=x_qT[dc * P:(dc + 1) * P,
                                               sqb * SKB:(sqb + 1) * SKB])
                        xq.append(t)
                    for hc in range(HC):
                        ps = proj_psum.tile([P, SKB], F32, tag="proj_kq")
                        for dc in range(DC):
                            nc.tensor.matmul(
                                ps[:],
                                w_sb["q"][:, dc, hc * P:(hc + 1) * P],
                                xq[dc][:],
                                start=(dc == 0), stop=(dc == DC - 1))
                        t = qT_pool.tile([P, SKB], F32R, tag="qT")
                        nc.vector.tensor_copy(t[:], ps[:])
                        qT_sb[hc][sqb] = t

            # ---- phase B: attention over query tiles ----
            with (
                tc.tile_pool(name="maskp", bufs=2) as mask_pool,
                tc.tile_pool(name="smp", bufs=3) as sm_pool,
                tc.tile_pool(name="pp", bufs=3) as p_pool,
                tc.tile_pool(name="ptsb", bufs=3) as pt_sb_pool,
                tc.tile_pool(name="denp", bufs=2) as den_pool,
                tc.tile_pool(name="osb", bufs=2) as o_sb_pool,
                tc.tile_pool(name="spsum", bufs=2, space="PSUM") as s_psum_pool,
                tc.tile_pool(name="ptpsum", bufs=2, space="PSUM") as pt_psum_pool,
                tc.tile_pool(name="opsum", bufs=2, space="PSUM") as o_psum_pool,
            ):
                for sqt in range(NSQT):
                    sqb, sqc = divmod(sqt, SKB // P)
                    m_sb = mask_pool.tile([P, SK_], BF16, tag="mask")
                    nc.sync.dma_start(
                        out=m_sb[:], in_=mask[sqt * P:(sqt + 1) * P, :])

                    o_ps = o_psum_pool.tile([P, H_], F32, tag="opsum")
                    den = den_pool.tile([P, NSKB + 2], F32, tag="den")

                    for skb in range(NSKB):
                        s_ps = s_psum_pool.tile([P, SKB], F32, tag="spsum")
                        for hc in range(HC):
                            nc.tensor.matmul(
                                s_ps[:],
                                qT_sb[hc][sqb][:, sqc * P:(sqc + 1) * P],
                                kT_sb[hc][skb][:],
                                start=(hc == 0), stop=(hc == HC - 1))
                        sm = sm_pool.tile([P, SKB], F32, tag="sm")
                        nc.vector.tensor_tensor(
                            sm[:], s_ps[:], m_sb[:, skb * SKB:(skb + 1) * SKB],
                            op=mybir.AluOpType.mult)
                        p_sb = p_pool.tile([P, SKB], BF16, tag="p")
                        nc.scalar.activation(
                            p_sb[:], sm[:], mybir.ActivationFunctionType.Exp,
                            scale=float(scale),
                            accum_out=den[:, skb:skb + 1])
                        pt_ps = pt_psum_pool.tile([P, SKB], BF16, tag="ptpsum")
                        for j in range(SKB // P):
                            nc.tensor.transpose(
                                pt_ps[:, j * P:(j + 1) * P],
                                p_sb[:, j * P:(j + 1) * P],
                                identity[:])
                        pt_sb = pt_sb_pool.tile([P, SKB], BF16, tag="ptsb")
                        nc.vector.tensor_copy(pt_sb[:], pt_ps[:])
                        for j in range(SKB // P):
                            kc = skb * (SKB // P) + j
                            nc.tensor.matmul(
                                o_ps[:],
                                pt_sb[:, j * P:(j + 1) * P],
                                v_sb[kc][:],
                                start=(skb == 0 and j == 0),
                                stop=(skb == NSKB - 1 and j == SKB // P - 1))

                    # normalize: out = o_ps / rowsum(P)
                    nc.vector.reduce_sum(
                        den[:, NSKB:NSKB + 1], den[:, 0:NSKB],
                        axis=mybir.AxisListType.X)
                    nc.vector.reciprocal(
                        den[:, NSKB + 1:NSKB + 2], den[:, NSKB:NSKB + 1])
                    # NB: tensor_scalar with an AP scalar reading PSUM
                    # directly hangs TRN2 here — bounce through SBUF.
                    o_tmp = o_sb_pool.tile([P, H_], F32, tag="otmp")
                    nc.scalar.copy(o_tmp[:], o_ps[:])
                    o_sb = o_sb_pool.tile([P, H_], F32, tag="osb")
                    nc.vector.tensor_scalar_mul(
                        o_sb[:], o_tmp[:], den[:, NSKB + 1:NSKB + 2])
                    nc.sync.dma_start(
                        out=out[sqt * P:(sqt + 1) * P, :], in_=o_sb[:])

    nc.compile()
    return nc


_COMPILED = None

# test-harness knobs (ignored in normal use)
TRACE = False
LAST_RESULT = None


def _get_compiled():
    global _COMPILED
    if _COMPILED is None:
        _COMPILED = build_attention_nc(SQL, S, DV, H, scale=1.0 / 16.0,
                                       num_devices=N_CORES)
    return _COMPILED


def kernel(x_q, x_k, x_v, mask, wq_w, wq_b, wk_w, wk_b, wv_w, wv_b):
    """Full inputs in, full output out. Shards across 8 NeuronCores."""
    nc = _get_compiled()

    x_q = np.asarray(x_q, dtype=np.float32)
    x_k = np.asarray(x_k, dtype=np.float32)
    x_v = np.asarray(x_v, dtype=np.float32)
    mask_bf = np.asarray(mask).astype(ml_dtypes.bfloat16)

    # transposed views (host-side layout prep)
    xqT = np.ascontiguousarray(np.swapaxes(x_q, 1, 2))  # [B, DV, S]
    xkT = np.ascontiguousarray(np.swapaxes(x_k, 1, 2))
    xvT = np.ascontiguousarray(np.swapaxes(x_v, 1, 2))
    wqT = np.ascontiguousarray(np.asarray(wq_w, dtype=np.float32).T)  # [DV,H]
    wkT = np.ascontiguousarray(np.asarray(wk_w, dtype=np.float32).T)
    wvT = np.ascontiguousarray(np.asarray(wv_w, dtype=np.float32).T)

    in_maps = []
    for c in range(N_CORES):
        b, half = divmod(c, CORES_PER_BATCH)
        q0 = half * SQL
        in_maps.append({
            "x_qT": np.ascontiguousarray(xqT[b][:, q0:q0 + SQL]),
            "x_kT": xkT[b],
            "x_vT": xvT[b],
            "mask": np.ascontiguousarray(mask_bf[b][q0:q0 + SQL]),
            "wqT": wqT,
            "wkT": wkT,
            "wvT": wvT,
        })

    global LAST_RESULT
    res = run_bass_kernel_spmd(nc, in_maps, core_ids=list(range(N_CORES)),
                               trace=TRACE)
    LAST_RESULT = res
    outs = res.results

    full = np.empty((B, S, H), dtype=np.float32)
    for c in range(N_CORES):
        b, half = divmod(c, CORES_PER_BATCH)
        q0 = half * SQL
        full[b, q0:q0 + SQL] = outs[c]["out"]
    return full



# revision 3
# speedup vs baseline: 1.3366x; 1.3366x over previous
"""Trainium2 Bass kernel for nn_AttentionHead (sparse attention, 8 cores).

Reference computation (per batch b):
    q = x_q @ wq^T ; k = x_k @ wk^T ; v = x_v @ wv^T          # [S, H]
    s = (q @ k^T) / sqrt(H)                                    # [S, S]
    s = where(mask == 0, 0, s)       # multiplicative 0/1 mask BEFORE softmax
    p = softmax(s, axis=-1)          # masked entries contribute exp(0)=1
    out = p @ v                                                # [S, H]

Sharding: 8 cores; core c -> batch c//2, query rows (c%2)*2048 ... +2048.
Each core computes k/v for its whole batch (duplicated within the pair),
so there are no collectives.

v2 design (vs the v1 baseline at 317us):
  * Scores are computed TRANSPOSED: for each 128-row sk chunk,
    sT[sk128, sq512] = K-chunk @ Q^T via matmul(lhsT=kT slice, rhs=qT block).
    After mask-mult + exp, pT[sk, sq] is already the lhsT layout the PV
    matmul needs -> the 512 PE transposes and 128 PSUM->SBUF bounce copies
    of v1 are gone.
  * Every matmul is bf16: HW shows fp32r matmuls at free-dim 256 run ~4x
    slower (390ns vs 107ns), and bf16 enables fast weight load. Accuracy
    emulated on CPU: rel err ~3.5e-3 (budget 2e-2).
  * Softmax denominator comes free from a ones-column appended to V
    (out[:, 256] = rowsum(P)) instead of ACT accumulator reads.
  * Mask is pre-cast to fp8e4m3 (0.0/1.0 exact) and pre-blocked on the host
    so each per-sqb slab is one fully-contiguous DMA.
  * Phase B is software-pipelined by one chunk so the PE stream interleaves
    scores(c+1) with PV(c) and never waits on the DVE/ACT chain.
"""

import numpy as np
import ml_dtypes

import concourse.bass as bass
import concourse.mybir as mybir
import concourse.tile as tile
from concourse import bacc
from concourse.bass_utils import run_bass_kernel_spmd

F32 = mybir.dt.float32
BF16 = mybir.dt.bfloat16
FP8 = mybir.dt.float8e4

# Full-problem constants
B, S, DV, H = 4, 4096, 1024, 256
N_CORES = 8
CORES_PER_BATCH = N_CORES // B
SQL = S // CORES_PER_BATCH  # query rows per core

P = 128
SKB = 512                    # block width (sq blocks and x blocks)
DC = DV // P                 # contraction chunks (8)
NSKB = S // SKB              # sk blocks of 512 (8)
NKC = S // P                 # sk chunks of 128 (32)
NSQB = SQL // SKB            # sq blocks of 512 (4)
HC = H // P                  # h chunks (2)
HP1 = H + 1                  # v columns incl. ones column (257)


def build_attention_nc(num_devices=1):
    nc = bacc.Bacc("TRN2", target_bir_lowering=False, debug=False,
                   num_devices=num_devices)

    # Host-blocked layouts: each leading index is one fully-contiguous DMA.
    x_q = nc.dram_tensor("x_q", [NSQB, P, DC, SKB], BF16, kind="ExternalInput").ap()
    x_k = nc.dram_tensor("x_k", [NSKB, P, DC, SKB], BF16, kind="ExternalInput").ap()
    x_v = nc.dram_tensor("x_v", [NSKB, P, DC, SKB], BF16, kind="ExternalInput").ap()
    mT = nc.dram_tensor("mT", [NSQB, P, NKC, SKB], FP8, kind="ExternalInput").ap()
    wq = nc.dram_tensor("wq", [P, DC, H], BF16, kind="ExternalInput").ap()
    wk = nc.dram_tensor("wk", [P, DC, H], BF16, kind="ExternalInput").ap()
    wv = nc.dram_tensor("wv", [P, DC, H], BF16, kind="ExternalInput").ap()
    out = nc.dram_tensor("out", [SQL, H], F32, kind="ExternalOutput").ap()

    scale = 1.0 / 16.0  # 1/sqrt(H)

    with tile.TileContext(nc) as tc:
        with (
            tc.tile_pool(name="weights", bufs=3) as w_pool,
            tc.tile_pool(name="maskp", bufs=NSQB) as mask_pool,
            tc.tile_pool(name="kT", bufs=HC * NSKB) as kT_pool,
            tc.tile_pool(name="qT", bufs=HC * NSQB) as qT_pool,
            tc.tile_pool(name="vsb", bufs=NKC) as v_pool,
        ):
            # ---- weights ----
            w_sb = {}
            for name, wT in (("q", wq), ("k", wk), ("v", wv)):
                t = w_pool.tile([P, DC, H], BF16, tag=f"w_{name}")
                nc.sync.dma_start(out=t[:], in_=wT)
                w_sb[name] = t

            kT_sb = [[None] * NSKB for _ in range(HC)]
            qT_sb = [[None] * NSQB for _ in range(HC)]
            v_sb = [None] * NKC

            # ---- phase A: projections (all-bf16 matmuls) ----
            with (
                tc.tile_pool(name="xkp", bufs=2) as xk_pool,
                tc.tile_pool(name="xvp", bufs=2) as xv_pool,
                tc.tile_pool(name="xqp", bufs=2) as xq_pool,
                tc.tile_pool(name="psA", bufs=2, space="PSUM") as psA,
                tc.tile_pool(name="psV", bufs=2, space="PSUM") as psV,
            ):
                for skb in range(NSKB):
                    xkt = xk_pool.tile([P, DC, SKB], BF16, tag="xk")
                    nc.sync.dma_start(out=xkt[:], in_=x_k[skb])
                    for hc in range(HC):
                        ps = psA.tile([P, SKB], F32, tag="psA")
                        for dc in range(DC):
                            nc.tensor.matmul(
                                ps[:],
                                w_sb["k"][:, dc, hc * P:(hc + 1) * P],
                                xkt[:, dc, :],
                                start=(dc == 0), stop=(dc == DC - 1))
                        t = kT_pool.tile([P, SKB], BF16, tag="kT")
                        nc.scalar.copy(t[:], ps[:])
                        kT_sb[hc][skb] = t

                    xvt = xv_pool.tile([P, DC, SKB], BF16, tag="xv")
                    nc.sync.dma_start(out=xvt[:], in_=x_v[skb])
                    for j in range(SKB // P):
                        kc = skb * (SKB // P) + j
                        ps = psV.tile([P, H], F32, tag="psV")
                        for dc in range(DC):
                            nc.tensor.matmul(
                                ps[:],
                                xvt[:, dc, j * P:(j + 1) * P],
                                w_sb["v"][:, dc, :],
                                start=(dc == 0), stop=(dc == DC - 1))
                        t = v_pool.tile([P, HP1], BF16, tag="v")
                        nc.vector.memset(t[:, H:HP1], 1.0)
                        nc.vector.tensor_copy(t[:, 0:H], ps[:])
                        v_sb[kc] = t

                for sqb in range(NSQB):
                    xqt = xq_pool.tile([P, DC, SKB], BF16, tag="xq")
                    nc.sync.dma_start(out=xqt[:], in_=x_q[sqb])
                    for hc in range(HC):
                        ps = psA.tile([P, SKB], F32, tag="psA")
                        for dc in range(DC):
                            nc.tensor.matmul(
                                ps[:],
                                w_sb["q"][:, dc, hc * P:(hc + 1) * P],
                                xqt[:, dc, :],
                                start=(dc == 0), stop=(dc == DC - 1))
                        t = qT_pool.tile([P, SKB], BF16, tag="qT")
                        nc.scalar.copy(t[:], ps[:])
                        qT_sb[hc][sqb] = t

            # ---- mask slab DMAs (needed from phase B; prefetch behind x) ----
            m_sb = []
            for sqb in range(NSQB):
                t = mask_pool.tile([P, NKC, SKB], FP8, tag="mask")
                nc.sync.dma_start(out=t[:], in_=mT[sqb])
                m_sb.append(t)

            # ---- phase B: attention, sq-block-major, sk-chunk pipeline ----
            with (
                tc.tile_pool(name="smp", bufs=3) as sm_pool,
                tc.tile_pool(name="pp", bufs=3) as p_pool,
                tc.tile_pool(name="osbp", bufs=2) as o_sb_pool,
                tc.tile_pool(name="ooutp", bufs=2) as o_out_pool,
                tc.tile_pool(name="denp", bufs=2) as den_pool,
                tc.tile_pool(name="spsum", bufs=2, space="PSUM") as s_psum,
                tc.tile_pool(name="opsum", bufs=4, space="PSUM") as o_psum,
            ):
                for sqb in range(NSQB):
                    o_ps = []
                    for j2 in range(SKB // P):
                        o_t = o_psum.tile([P, HP1], F32, tag="opsum")
                        o_ps.append(o_t)

                    prev = None
                    for kc in range(NKC + 1):
                        if kc < NKC:
                            skb, j = divmod(kc, SKB // P)
                            sp = s_psum.tile([P, SKB], F32, tag="spsum")
                            nc.tensor.matmul(
                                sp[:],
                                kT_sb[0][skb][:, j * P:(j + 1) * P],
                                qT_sb[0][sqb][:],
                                start=True, stop=False)
                            nc.tensor.matmul(
                                sp[:],
                                kT_sb[1][skb][:, j * P:(j + 1) * P],
                                qT_sb[1][sqb][:],
                                start=False, stop=True)
                        else:
                            sp = None
                        if prev is not None:
                            pkc, psp = prev
                            sm = sm_pool.tile([P, SKB], F32, tag="sm")
                            nc.vector.tensor_tensor(
                                sm[:], psp[:], m_sb[sqb][:, pkc, :],
                                op=mybir.AluOpType.mult)
                            pT = p_pool.tile([P, SKB], BF16, tag="p")
                            nc.scalar.activation(
                                pT[:], sm[:],
                                mybir.ActivationFunctionType.Exp,
                                scale=scale)
                            for j2 in range(SKB // P):
                                nc.tensor.matmul(
                                    o_ps[j2][:],
                                    pT[:, j2 * P:(j2 + 1) * P],
                                    v_sb[pkc][:],
                                    start=(pkc == 0),
                                    stop=(pkc == NKC - 1))
                        prev = (kc, sp) if sp is not None else None

                    # epilogue: normalize by the ones-column rowsum and store
                    for j2 in range(SKB // P):
                        osb = o_sb_pool.tile([P, HP1], F32, tag="osb")
                        nc.scalar.copy(osb[:], o_ps[j2][:])
                        den = den_pool.tile([P, 1], F32, tag="den")
                        nc.vector.reciprocal(den[:], osb[:, H:HP1])
                        oout = o_out_pool.tile([P, H], F32, tag="oout")
                        nc.vector.tensor_scalar_mul(
                            oout[:], osb[:, 0:H], den[:])
                        r0 = sqb * SKB + j2 * P
                        nc.sync.dma_start(out=out[r0:r0 + P, :], in_=oout[:])

    nc.compile()
    return nc


_COMPILED = None

# test-harness knobs (ignored in normal use)
TRACE = False
LAST_RESULT = None


def _get_compiled():
    global _COMPILED
    if _COMPILED is None:
        _COMPILED = build_attention_nc(num_devices=N_CORES)
    return _COMPILED


def _block_xT(xT):
    """[DV, W] f32 -> [W//SKB, P, DC, SKB] bf16 with
    blocks[wb, p, dc, w] = xT[dc*P + p, wb*SKB + w]."""
    W = xT.shape[1]
    b = xT.reshape(DC, P, W // SKB, SKB).transpose(2, 1, 0, 3)
    return np.ascontiguousarray(b.astype(ml_dtypes.bfloat16))


def _block_w(w):
    """[H, DV] f32 -> [P, DC, H] bf16 with blocks[p, dc, h] = w[h, dc*P+p]."""
    b = w.T.reshape(DC, P, H).transpose(1, 0, 2)
    return np.ascontiguousarray(b.astype(ml_dtypes.bfloat16))


def _block_maskT(maskT):
    """[S, SQL] -> [NSQB, P, NKC, SKB] fp8 with
    blocks[sqb, p, c, w] = maskT[c*P + p, sqb*SKB + w]."""
    b = maskT.reshape(NKC, P, NSQB, SKB).transpose(2, 1, 0, 3)
    return np.ascontiguousarray(b.astype(ml_dtypes.float8_e4m3))


def kernel(x_q, x_k, x_v, mask, wq_w, wq_b, wk_w, wk_b, wv_w, wv_b):
    """Full inputs in, full output out. Shards across 8 NeuronCores."""
    nc = _get_compiled()

    x_q = np.asarray(x_q, dtype=np.float32)
    x_k = np.asarray(x_k, dtype=np.float32)
    x_v = np.asarray(x_v, dtype=np.float32)
    mask = np.asarray(mask)

    wqb = _block_w(np.asarray(wq_w, dtype=np.float32))
    wkb = _block_w(np.asarray(wk_w, dtype=np.float32))
    wvb = _block_w(np.asarray(wv_w, dtype=np.float32))

    in_maps = []
    for c in range(N_CORES):
        b, half = divmod(c, CORES_PER_BATCH)
        q0 = half * SQL
        xqT = x_q[b][q0:q0 + SQL].T            # [DV, SQL]
        xkT = x_k[b].T                         # [DV, S]
        xvT = x_v[b].T
        maskT = mask[b][q0:q0 + SQL].T         # [S(k), SQL(q)]
        in_maps.append({
            "x_q": _block_xT(xqT),
            "x_k": _block_xT(xkT),
            "x_v": _block_xT(xvT),
            "mT": _block_maskT(maskT),
            "wq": wqb,
            "wk": wkb,
            "wv": wvb,
        })

    global LAST_RESULT
    res = run_bass_kernel_spmd(nc, in_maps, core_ids=list(range(N_CORES)),
                               trace=TRACE)
    LAST_RESULT = res
    outs = res.results

    full = np.empty((B, S, H), dtype=np.float32)
    for c in range(N_CORES):
        b, half = divmod(c, CORES_PER_BATCH)
        q0 = half * SQL
        full[b, q0:q0 + SQL] = outs[c]["out"]
    return full


# revision 6
# speedup vs baseline: 1.3835x; 1.0350x over previous
"""Trainium2 Bass kernel for nn_AttentionHead (sparse attention, 8 cores).

Reference computation (per batch b):
    q = x_q @ wq^T ; k = x_k @ wk^T ; v = x_v @ wv^T          # [S, H]
    s = (q @ k^T) / sqrt(H)                                    # [S, S]
    s = where(mask == 0, 0, s)       # multiplicative 0/1 mask BEFORE softmax
    p = softmax(s, axis=-1)          # masked entries contribute exp(0)=1
    out = p @ v                                                # [S, H]

Sharding: 8 cores; core c -> batch c//2, query rows (c%2)*2048 ... +2048.
Each core computes k/v for its whole batch (duplicated within the pair),
so there are no collectives.

v2 design (vs the v1 baseline at 317us):
  * Scores are computed TRANSPOSED: for each 128-row sk chunk,
    sT[sk128, sq512] = K-chunk @ Q^T via matmul(lhsT=kT slice, rhs=qT block).
    After mask-mult + exp, pT[sk, sq] is already the lhsT layout the PV
    matmul needs -> the 512 PE transposes and 128 PSUM->SBUF bounce copies
    of v1 are gone.
  * Every matmul is bf16: HW shows fp32r matmuls at free-dim 256 run ~4x
    slower (390ns vs 107ns), and bf16 enables fast weight load. Accuracy
    emulated on CPU: rel err ~3.5e-3 (budget 2e-2).
  * Softmax denominator comes free from a ones-column appended to V
    (out[:, 256] = rowsum(P)) instead of ACT accumulator reads.
  * Mask is pre-cast to fp8e4m3 (0.0/1.0 exact) and pre-blocked on the host
    so each per-sqb slab is one fully-contiguous DMA.
  * Phase B is software-pipelined by one chunk so the PE stream interleaves
    scores(c+1) with PV(c) and never waits on the DVE/ACT chain.
"""

import numpy as np
import ml_dtypes

import concourse.bass as bass
import concourse.mybir as mybir
import concourse.tile as tile
from concourse import bacc
from concourse.bass_utils import run_bass_kernel_spmd

F32 = mybir.dt.float32
BF16 = mybir.dt.bfloat16
FP8 = mybir.dt.float8e4

# Full-problem constants
B, S, DV, H = 4, 4096, 1024, 256
N_CORES = 8
CORES_PER_BATCH = N_CORES // B
SQL = S // CORES_PER_BATCH  # query rows per core

P = 128
SKB = 512                    # block width (sq blocks and x blocks)
DC = DV // P                 # contraction chunks (8)
NSKB = S // SKB              # sk blocks of 512 (8)
NKC = S // P                 # sk chunks of 128 (32)
NSQB = SQL // SKB            # sq blocks of 512 (4)
HC = H // P                  # h chunks (2)
HP1 = H + 1                  # v columns incl. ones column (257)


def build_attention_nc(num_devices=1):
    nc = bacc.Bacc("TRN2", target_bir_lowering=False, debug=False,
                   num_devices=num_devices)

    # Host-blocked layouts: each leading index is one fully-contiguous DMA.
    x_q = nc.dram_tensor("x_q", [NSQB, P, DC, SKB], BF16, kind="ExternalInput").ap()
    x_k = nc.dram_tensor("x_k", [NSKB, P, DC, SKB], BF16, kind="ExternalInput").ap()
    x_v = nc.dram_tensor("x_v", [NSKB, P, DC, SKB], BF16, kind="ExternalInput").ap()
    mT = nc.dram_tensor("mT", [NSQB, P, NKC, SKB], FP8, kind="ExternalInput").ap()
    wq = nc.dram_tensor("wq", [P, DC, H], BF16, kind="ExternalInput").ap()
    wk = nc.dram_tensor("wk", [P, DC, H], BF16, kind="ExternalInput").ap()
    wv = nc.dram_tensor("wv", [P, DC, H], BF16, kind="ExternalInput").ap()
    out = nc.dram_tensor("out", [SQL, H], F32, kind="ExternalOutput").ap()

    scale = 1.0 / 16.0  # 1/sqrt(H)

    with tile.TileContext(nc) as tc:
        with (
            tc.tile_pool(name="weights", bufs=3) as w_pool,
            tc.tile_pool(name="maskp", bufs=NSQB) as mask_pool,
            tc.tile_pool(name="kT", bufs=HC * NSKB) as kT_pool,
            tc.tile_pool(name="qT", bufs=HC * NSQB) as qT_pool,
            tc.tile_pool(name="vsb", bufs=NKC) as v_pool,
        ):
            # ---- weights ----
            w_sb = {}
            for name, wT in (("q", wq), ("k", wk), ("v", wv)):
                t = w_pool.tile([P, DC, H], BF16, tag=f"w_{name}")
                nc.sync.dma_start(out=t[:], in_=wT)
                w_sb[name] = t

            kT_sb = [[None] * NSKB for _ in range(HC)]
            qT_sb = [[None] * NSQB for _ in range(HC)]
            v_sb = [None] * NKC

            # ---- phase A: projections (all-bf16 matmuls) ----
            m_sb = [None] * NSQB
            with (
                tc.tile_pool(name="xkp", bufs=3) as xk_pool,
                tc.tile_pool(name="xvp", bufs=2) as xv_pool,
                tc.tile_pool(name="xqp", bufs=2) as xq_pool,
                tc.tile_pool(name="psA", bufs=2, space="PSUM") as psA,
                tc.tile_pool(name="psV", bufs=2, space="PSUM") as psV,
            ):
                for skb in range(NSKB):
                    xkt = xk_pool.tile([P, DC, SKB], BF16, tag="xk")
                    nc.sync.dma_start(out=xkt[:], in_=x_k[skb])
                    if skb == 1:
                        # mask slab 0 is needed right at phase-B start; emit
                        # its DMA early so it stripes into the stream before
                        # the tail of the x loads.
                        t = mask_pool.tile([P, NKC, SKB], FP8, tag="mask")
                        nc.sync.dma_start(out=t[:], in_=mT[0])
                        m_sb[0] = t
                    for hc in range(HC):
                        ps = psA.tile([P, SKB], F32, tag="psA")
                        for dc in range(DC):
                            nc.tensor.matmul(
                                ps[:],
                                w_sb["k"][:, dc, hc * P:(hc + 1) * P],
                                xkt[:, dc, :],
                                start=(dc == 0), stop=(dc == DC - 1))
                        t = kT_pool.tile([P, SKB], BF16, tag="kT")
                        nc.scalar.copy(t[:], ps[:])
                        kT_sb[hc][skb] = t

                    xvt = xv_pool.tile([P, DC, SKB], BF16, tag="xv")
                    nc.sync.dma_start(out=xvt[:], in_=x_v[skb])
                    for j in range(SKB // P):
                        kc = skb * (SKB // P) + j
                        ps = psV.tile([P, H], F32, tag="psV")
                        for dc in range(DC):
                            nc.tensor.matmul(
                                ps[:],
                                xvt[:, dc, j * P:(j + 1) * P],
                                w_sb["v"][:, dc, :],
                                start=(dc == 0), stop=(dc == DC - 1))
                        t = v_pool.tile([P, HP1], BF16, tag="v")
                        nc.vector.memset(t[:, H:HP1], 1.0)
                        nc.vector.tensor_copy(t[:, 0:H], ps[:])
                        v_sb[kc] = t

                for sqb in range(NSQB):
                    xqt = xq_pool.tile([P, DC, SKB], BF16, tag="xq")
                    nc.sync.dma_start(out=xqt[:], in_=x_q[sqb])
                    for hc in range(HC):
                        ps = psA.tile([P, SKB], F32, tag="psA")
                        for dc in range(DC):
                            nc.tensor.matmul(
                                ps[:],
                                w_sb["q"][:, dc, hc * P:(hc + 1) * P],
                                xqt[:, dc, :],
                                start=(dc == 0), stop=(dc == DC - 1))
                        t = qT_pool.tile([P, SKB], BF16, tag="qT")
                        nc.scalar.copy(t[:], ps[:])
                        qT_sb[hc][sqb] = t

            # ---- remaining mask slabs (needed at ~+28us intervals) ----
            for sqb in range(1, NSQB):
                t = mask_pool.tile([P, NKC, SKB], FP8, tag="mask")
                nc.sync.dma_start(out=t[:], in_=mT[sqb])
                m_sb[sqb] = t

            # ---- phase B: attention, sq-block-major, sk-chunk pipeline ----
            with (
                tc.tile_pool(name="smp", bufs=4) as sm_pool,
                tc.tile_pool(name="pp", bufs=4) as p_pool,
                tc.tile_pool(name="osbp", bufs=2) as o_sb_pool,
                tc.tile_pool(name="ooutp", bufs=2) as o_out_pool,
                tc.tile_pool(name="denp", bufs=2) as den_pool,
                tc.tile_pool(name="spsum", bufs=3, space="PSUM") as s_psum,
                tc.tile_pool(name="opsum", bufs=4, space="PSUM") as o_psum,
            ):
                for sqb in range(NSQB):
                    o_ps = []
                    for j2 in range(SKB // P):
                        o_t = o_psum.tile([P, HP1], F32, tag="opsum")
                        o_ps.append(o_t)

                    prev = None
                    for kc in range(NKC + 1):
                        if kc < NKC:
                            skb, j = divmod(kc, SKB // P)
                            sp = s_psum.tile([P, SKB], F32, tag="spsum")
                            nc.tensor.matmul(
                                sp[:],
                                kT_sb[0][skb][:, j * P:(j + 1) * P],
                                qT_sb[0][sqb][:],
                                start=True, stop=False)
                            nc.tensor.matmul(
                                sp[:],
                                kT_sb[1][skb][:, j * P:(j + 1) * P],
                                qT_sb[1][sqb][:],
                                start=False, stop=True)
                        else:
                            sp = None
                        if prev is not None:
                            pkc, psp = prev
                            sm = sm_pool.tile([P, SKB], F32, tag="sm")
                            nc.vector.tensor_tensor(
                                sm[:], psp[:], m_sb[sqb][:, pkc, :],
                                op=mybir.AluOpType.mult)
                            pT = p_pool.tile([P, SKB], BF16, tag="p")
                            nc.scalar.activation(
                                pT[:], sm[:],
                                mybir.ActivationFunctionType.Exp,
                                scale=scale)
                            for j2 in range(SKB // P):
                                nc.tensor.matmul(
                                    o_ps[j2][:],
                                    pT[:, j2 * P:(j2 + 1) * P],
                                    v_sb[pkc][:],
                                    start=(pkc == 0),
                                    stop=(pkc == NKC - 1))
                        prev = (kc, sp) if sp is not None else None

                    # epilogue: normalize by the ones-column rowsum and store
                    for j2 in range(SKB // P):
                        osb = o_sb_pool.tile([P, HP1], F32, tag="osb")
                        nc.scalar.copy(osb[:], o_ps[j2][:])
                        den = den_pool.tile([P, 1], F32, tag="den")
                        nc.vector.reciprocal(den[:], osb[:, H:HP1])
                        oout = o_out_pool.tile([P, H], F32, tag="oout")
                        nc.vector.tensor_scalar_mul(
                            oout[:], osb[:, 0:H], den[:])
                        r0 = sqb * SKB + j2 * P
                        nc.sync.dma_start(out=out[r0:r0 + P, :], in_=oout[:])

    nc.compile()
    return nc


_COMPILED = None

# test-harness knobs (ignored in normal use)
TRACE = False
LAST_RESULT = None


def _get_compiled():
    global _COMPILED
    if _COMPILED is None:
        _COMPILED = build_attention_nc(num_devices=N_CORES)
    return _COMPILED


def _block_xT(xT):
    """[DV, W] f32 -> [W//SKB, P, DC, SKB] bf16 with
    blocks[wb, p, dc, w] = xT[dc*P + p, wb*SKB + w]."""
    W = xT.shape[1]
    b = xT.reshape(DC, P, W // SKB, SKB).transpose(2, 1, 0, 3)
    return np.ascontiguousarray(b.astype(ml_dtypes.bfloat16))


def _block_w(w):
    """[H, DV] f32 -> [P, DC, H] bf16 with blocks[p, dc, h] = w[h, dc*P+p]."""
    b = w.T.reshape(DC, P, H).transpose(1, 0, 2)
    return np.ascontiguousarray(b.astype(ml_dtypes.bfloat16))


def _block_maskT(maskT):
    """[S, SQL] -> [NSQB, P, NKC, SKB] fp8 with
    blocks[sqb, p, c, w] = maskT[c*P + p, sqb*SKB + w]."""
    b = maskT.reshape(NKC, P, NSQB, SKB).transpose(2, 1, 0, 3)
    return np.ascontiguousarray(b.astype(ml_dtypes.float8_e4m3))


def kernel(x_q, x_k, x_v, mask, wq_w, wq_b, wk_w, wk_b, wv_w, wv_b):
    """Full inputs in, full output out. Shards across 8 NeuronCores."""
    nc = _get_compiled()

    x_q = np.asarray(x_q, dtype=np.float32)
    x_k = np.asarray(x_k, dtype=np.float32)
    x_v = np.asarray(x_v, dtype=np.float32)
    mask = np.asarray(mask)

    wqb = _block_w(np.asarray(wq_w, dtype=np.float32))
    wkb = _block_w(np.asarray(wk_w, dtype=np.float32))
    wvb = _block_w(np.asarray(wv_w, dtype=np.float32))

    in_maps = []
    for c in range(N_CORES):
        b, half = divmod(c, CORES_PER_BATCH)
        q0 = half * SQL
        xqT = x_q[b][q0:q0 + SQL].T            # [DV, SQL]
        xkT = x_k[b].T                         # [DV, S]
        xvT = x_v[b].T
        maskT = mask[b][q0:q0 + SQL].T         # [S(k), SQL(q)]
        in_maps.append({
            "x_q": _block_xT(xqT),
            "x_k": _block_xT(xkT),
            "x_v": _block_xT(xvT),
            "mT": _block_maskT(maskT),
            "wq": wqb,
            "wk": wkb,
            "wv": wvb,
        })

    global LAST_RESULT
    res = run_bass_kernel_spmd(nc, in_maps, core_ids=list(range(N_CORES)),
                               trace=TRACE)
    LAST_RESULT = res
    outs = res.results

    full = np.empty((B, S, H), dtype=np.float32)
    for c in range(N_CORES):
        b, half = divmod(c, CORES_PER_BATCH)
        q0 = half * SQL
        full[b, q0:q0 + SQL] = outs[c]["out"]
    return full
